# revision 1
# baseline (speedup 1.0000x reference)
"""Multi-head self-attention (RoPE, causal) on 8 Trainium2 NeuronCores.

Sharding: tensor-parallel over heads. Each core owns 2 of 16 heads:
  - QKV projections column-sharded (each core computes its 128 features)
  - attention per (batch, head) pair fully on-core, scores kept in the
    transposed orientation [tk, tq] so softmax needs no PE transposes:
    exp on ACT, denominator via a ones-row appended to V, causal handled
    block-wise + a triangular mask on diagonal blocks
  - AllToAll switches from head-sharding to token-sharding (4MB/core)
  - output projection token-sharded, output written in natural layout

dtypes: fp32r (TF32-like, full PE rate at N>=512) everywhere except the
softmax probabilities / V operand of the PV matmul, which are bf16.
"""

import numpy as np
import ml_dtypes

import concourse.bacc as bacc
import concourse.mybir as mybir
import concourse.tile as tile
from concourse import bass_utils
from concourse.masks import make_identity

F32 = mybir.dt.float32
F32R = mybir.dt.float32r
BF16 = mybir.dt.bfloat16

B, T, D = 4, 2048, 1024
H, DH = 16, 64
N_CORES = 8
HPC = H // N_CORES            # heads per core = 2
EC = HPC * DH                 # feature slice per core = 128
NT = B * T                    # 8192 tokens
TPC = NT // N_CORES           # 1024 tokens per core
THETA = 10000.0

_CACHE = {}
last_results = None  # BassKernelResults of the most recent run (for profiling)


def _build_program():
    nc = bacc.Bacc("TRN2", debug=False, target_bir_lowering=False,
                   num_devices=N_CORES)

    xt_d = nc.dram_tensor("xt", [128, 8, NT], BF16, kind="ExternalInput")
    wq_d = nc.dram_tensor("wq", [128, 8, EC], BF16, kind="ExternalInput")
    wk_d = nc.dram_tensor("wk", [128, 8, EC], BF16, kind="ExternalInput")
    wv_d = nc.dram_tensor("wv", [128, 8, EC], BF16, kind="ExternalInput")
    wo_d = nc.dram_tensor("wo", [128, 8, D], BF16, kind="ExternalInput")
    cos_d = nc.dram_tensor("cosb", [128, T], F32, kind="ExternalInput")
    sin_d = nc.dram_tensor("sinb", [128, T], F32, kind="ExternalInput")
    rotm_d = nc.dram_tensor("rotm", [128, 128], F32R, kind="ExternalInput")
    tri_d = nc.dram_tensor("trimask", [128, 128], BF16, kind="ExternalInput")
    y_d = nc.dram_tensor("y", [TPC, D], F32, kind="ExternalOutput")

    NB = T // 128      # 16 tk blocks per batch
    NCHUNK = NT // 512  # 16 phase-1 chunks

    with tile.TileContext(nc) as tc:
        with (
            tc.tile_pool(name="consts", bufs=1) as consts,
            tc.tile_pool(name="wpool", bufs=1) as wpool,
            tc.tile_pool(name="big", bufs=1) as big,
            tc.tile_pool(name="xp", bufs=2) as xp,
            tc.tile_pool(name="stage", bufs=2) as stage,
            tc.tile_pool(name="expp", bufs=4) as expp,
            tc.tile_pool(name="outp", bufs=2) as outp,
            tc.tile_pool(name="psA", bufs=1, space="PSUM") as psA,
            tc.tile_pool(name="psB", bufs=1, space="PSUM") as psB,
            tc.tile_pool(name="pvA", bufs=2, space="PSUM") as pvA,
            tc.tile_pool(name="pvB", bufs=2, space="PSUM") as pvB,
            tc.tile_pool(name="dram", bufs=2, space="DRAM") as dram,
        ):
            # ---- constants ----
            cos_sb = consts.tile([128, T], F32)
            sin_sb = consts.tile([128, T], F32)
            rotm_sb = consts.tile([128, 128], F32R)
            tri_sb = consts.tile([128, 128], BF16)
            ident_sb = consts.tile([128, 128], F32)
            nc.sync.dma_start(cos_sb[:], cos_d[:, :])
            nc.sync.dma_start(sin_sb[:], sin_d[:, :])
            nc.sync.dma_start(rotm_sb[:], rotm_d[:, :])
            nc.sync.dma_start(tri_sb[:], tri_d[:, :])
            make_identity(nc, ident_sb[:])

            wq_sb = consts.tile([128, 8, EC], BF16)
            wk_sb = consts.tile([128, 8, EC], BF16)
            wv_sb = consts.tile([128, 8, EC], BF16)
            nc.sync.dma_start(wq_sb[:], wq_d[:, :, :])
            nc.sync.dma_start(wk_sb[:], wk_d[:, :, :])
            nc.sync.dma_start(wv_sb[:], wv_d[:, :, :])

            # ---- persistent tensors ----
            qT = big.tile([128, NT], F32R, tag="qT")
            kT = big.tile([128, NT], F32R, tag="kT")
            # V per (pair, tk-block): [tk=128, 65] with ones in col 64
            vext = big.tile([128, HPC * B, NB, 65], BF16, tag="vext")
            nc.vector.memset(vext[:, :, :, 64], 1.0)

            a2a_in1 = dram.tile([N_CORES, 128, 768], BF16)
            a2a_out1 = dram.tile([N_CORES, 128, 768], BF16)
            a2a_in2 = dram.tile([N_CORES, 128, 256], BF16)
            a2a_out2 = dram.tile([N_CORES, 128, 256], BF16)

            # ================= Phase 1: QKV projections + RoPE =============
            def do_chunk(ci):
                t0 = 512 * ci
                bb = t0 // T
                s0 = t0 % T
                xt = xp.tile([128, 8, 512], BF16, tag="x")
                nc.sync.dma_start(xt[:], xt_d[:, :, t0:t0 + 512])

                # pipelined: proj(q) -> ACTcopy(q) -> proj(k) -> rot(q) ->
                # ACTcopy(k) -> proj(v) -> rot(k) -> ACTcopy(v) -> vtrans
                # so PE never sits behind an ACT drain.
                def _proj(w_sb, nm, pool):
                    pt = pool.tile([128, 1024], F32, tag="s", name="p" + nm)
                    pt = pt[:, 0:512]
                    for ko in range(8):
                        nc.tensor.matmul(pt, w_sb[:, ko, :], xt[:, ko, :],
                                         start=(ko == 0), stop=(ko == 7))
                    return pt

                def _rot(raw, nm, pool):
                    rot = pool.tile([128, 1024], F32, tag="s", name="r" + nm)
                    rot = rot[:, 0:512]
                    nc.tensor.matmul(rot, rotm_sb[:], raw[:],
                                     start=True, stop=True)
                    return rot

                def _rope_combine(raw, rot, dest):
                    t1 = stage.tile([128, 512], F32, tag="t1")
                    nc.vector.tensor_tensor(
                        t1[:], raw[:], cos_sb[:, s0:s0 + 512],
                        mybir.AluOpType.mult)
                    t2 = stage.tile([128, 512], F32, tag="t2")
                    nc.vector.tensor_tensor(
                        t2[:], rot[:], sin_sb[:, s0:s0 + 512],
                        mybir.AluOpType.mult)
                    nc.vector.tensor_tensor(
                        dest[:, t0:t0 + 512], t1[:], t2[:],
                        mybir.AluOpType.add)

                pq = _proj(wq_sb, "q", psA)
                rawq = stage.tile([128, 512], F32R, tag="rawq")
                nc.scalar.copy(rawq[:], pq)
                pk = _proj(wk_sb, "k", psB)
                rotq = _rot(rawq, "q", psA)
                rawk = stage.tile([128, 512], F32R, tag="rawk")
                nc.scalar.copy(rawk[:], pk)
                pv_ = _proj(wv_sb, "v", psB)
                rotk = _rot(rawk, "k", psA)
                vraw = stage.tile([128, 512], F32, tag="vraw")
                nc.scalar.copy(vraw[:], pv_)
                _rope_combine(rawq, rotq, qT)
                _rope_combine(rawk, rotk, kT)
                for h in range(HPC):
                    pair = bb * HPC + h
                    for bi in range(4):
                        jg = s0 // 128 + bi
                        tp = (psB if bi % 2 else psA).tile(
                            [128, 1024], F32, tag="s", name="vtr")[:, 0:64]
                        nc.tensor.transpose(
                            tp,
                            vraw[64 * h:64 * h + 64,
                                 128 * bi:128 * bi + 128],
                            ident_sb[64 * h:64 * h + 64,
                                     64 * h:64 * h + 64])
                        nc.vector.tensor_copy(
                            vext[:, pair, jg, 0:64], tp)

            # ================= Phase 2: attention ==========================
            # Two heads of the same batch run as interleaved pipeline
            # streams: ACT-exp latency of one stream hides behind PE work
            # of the other.
            def do_attn(bb):
                tb0 = bb * T
                qs = [qT[64 * hh:64 * hh + 64, tb0:tb0 + T] for hh in range(2)]
                ks = [kT[64 * hh:64 * hh + 64, tb0:tb0 + T] for hh in range(2)]
                spools = [psA, psB]
                vpools = [pvA, pvB]
                for c2 in range(2):
                    jmax = 8 * (c2 + 1)
                    pvt = [[vpools[hh].tile(
                        [65, 512], F32, tag="pv",
                        name=f"pv_{bb}_{hh}_{c2}_{hf}") for hf in range(2)]
                        for hh in range(2)]

                    def _scores_pair(j):
                        # both heads' score matmuls, issued alternating so
                        # the two K=64 row-strips (partitions 0-63 / 64-127)
                        # execute concurrently in the PE array.
                        spts = [spools[hh].tile(
                            [128, 1024], F32, tag="s",
                            name=f"s_{bb}_{hh}_{c2}_{j}") for hh in range(2)]
                        for hf in range(2):
                            cl0 = 1024 * c2 + 512 * hf
                            if cl0 + 512 <= 128 * j:
                                continue
                            w = cl0 + 512 - max(cl0, 128 * j)
                            N = 512 if w == 512 else max(256, w)
                            st = cl0 + 512 - N
                            for hh in range(2):
                                nc.tensor.matmul(
                                    spts[hh][:, st - 1024 * c2:
                                             st - 1024 * c2 + N],
                                    ks[hh][:, 128 * j:128 * j + 128],
                                    qs[hh][:, st:st + N],
                                    start=True, stop=True)
                        return spts

                    def _pv(j, exs):
                        lo = max(0, 128 * j - 1024 * c2)
                        for hh in range(2):
                            vt = vext[:, bb * HPC + hh, j, :]
                            for hf in range(2):
                                h0 = 512 * hf
                                a = max(h0, lo)
                                if a < h0 + 512:
                                    last_j = min(jmax - 1,
                                                 8 * c2 + 4 * hf + 3)
                                    nc.tensor.matmul(
                                        pvt[hh][hf][:, a - h0:512],
                                        vt, exs[hh][:, a:h0 + 512],
                                        start=(j == 0), stop=(j == last_j))

                    # software pipeline with one-iteration PV delay: the PE
                    # always has ready work (PV of j-1) at its queue head
                    # while ACT computes exp(j).
                    spt = _scores_pair(0)
                    prev = None
                    for j in range(jmax):
                        lo = max(0, 128 * j - 1024 * c2)
                        exs = []
                        for hh in range(2):
                            ex = expp.tile([128, 1024], BF16, tag="e",
                                           name=f"e_{hh}")
                            nc.scalar.activation(
                                ex[:, lo:1024], spt[hh][:, lo:1024],
                                mybir.ActivationFunctionType.Exp, scale=0.125)
                            exs.append(ex)
                        if prev is not None:
                            _pv(prev[0], prev[1])
                        if j + 1 < jmax:
                            spt = _scores_pair(j + 1)
                        for hh in range(2):
                            if 128 * j >= 1024 * c2:
                                nc.vector.tensor_tensor(
                                    exs[hh][:, lo:lo + 128],
                                    exs[hh][:, lo:lo + 128],
                                    tri_sb[:], mybir.AluOpType.mult)
                        prev = (j, exs)
                    _pv(prev[0], prev[1])
                    # normalize + ship to a2a_in.  Copy psum out first
                    # (ACT) so the pv slots free up for the next chunk.
                    for hh in range(2):
                        dnm = outp.tile([33, 512], F32, tag="dnm")
                        unn = [None, None]
                        for hf in range(2):
                            nc.vector.tensor_copy(
                                dnm[32 * hf:32 * hf + 1, :],
                                pvt[hh][hf][64:65, :])
                            unn[hf] = outp.tile([64, 512], BF16,
                                                tag=f"unn{hf}",
                                                name=f"unn{hf}")
                            nc.scalar.copy(unn[hf][:], pvt[hh][hf][0:64, :])
                        rec = outp.tile([33, 512], F32, tag="rec")
                        nc.vector.reciprocal(rec[:], dnm[:])
                        rscr = dram.tile([2, 512], F32, tag="rscr",
                                         name="rscr")
                        for hf in range(2):
                            nc.sync.dma_start(rscr[hf:hf + 1, :],
                                              rec[32 * hf:32 * hf + 1, :])
                        for hf in range(2):
                            recb = outp.tile([64, 512], F32, tag="recb")
                            nc.sync.dma_start(
                                recb[:],
                                rscr[hf:hf + 1, :].to_broadcast((64, 512)))
                            ao = outp.tile([64, 512], BF16, tag="ao")
                            nc.vector.tensor_tensor(
                                ao[:], unn[hf][:], recb[:],
                                mybir.AluOpType.mult)
                            # group 1 = batches 0-2 (768 tokens/dest),
                            # group 2 = batch 3 (256 tokens/dest)
                            if bb < 3:
                                grp, base, W = a2a_in1, 0, 768
                            else:
                                grp, base, W = a2a_in2, 6144, 256
                            tt = 2048 * bb + 1024 * c2 + 512 * hf - base
                            off = 0
                            while off < 512:
                                dd = (tt + off) // W
                                col = (tt + off) % W
                                w = min(512 - off, W - col)
                                nc.sync.dma_start(
                                    grp[dd, 64 * hh:64 * hh + 64,
                                        col:col + w],
                                    ao[:, off:off + w])
                                off += w

            def do_oproj(g, oall_g, row0, ntb):
                # y rows [row0, row0 + 128*ntb) from group-g tokens
                for eo in range(2):
                    wo_sb = wpool.tile([128, 8, 512], BF16, tag="wo",
                                       name=f"wo_{g}_{eo}")
                    nc.sync.dma_start(wo_sb[:],
                                      wo_d[:, :, 512 * eo:512 * eo + 512])
                    for tb in range(ntb):
                        ot = (psB if (tb + eo) % 2 else psA).tile(
                            [128, 1024], F32, tag="s", name="ot")[:, 0:512]
                        for ec in range(8):
                            nc.tensor.matmul(
                                ot, oall_g[:, ec, 128 * tb:128 * tb + 128],
                                wo_sb[:, ec, :],
                                start=(ec == 0), stop=(ec == 7))
                        ys = outp.tile([128, 512], F32, tag="y")
                        nc.scalar.copy(ys[:], ot)
                        nc.sync.dma_start(
                            y_d[row0 + 128 * tb:row0 + 128 * tb + 128,
                                512 * eo:512 * eo + 512], ys[:])

            # interleave phase 1 and attention per batch; group-0 A2A and
            # its output projection overlap batches 2-3.
            rg = [list(range(N_CORES))]
            for bb in range(3):
                for ci in range(4 * bb, 4 * bb + 4):
                    do_chunk(ci)
                do_attn(bb)
            nc.gpsimd.collective_compute(
                "AllToAll", mybir.AluOpType.bypass, replica_groups=rg,
                ins=[a2a_in1.opt()], outs=[a2a_out1.opt()])
            oall1 = wpool.tile([128, 8, 768], BF16, tag="oall1")
            nc.sync.dma_start(oall1[:],
                              a2a_out1[:].rearrange("s p t -> p s t"))
            for ci in range(12, 16):
                do_chunk(ci)
            do_attn(3)
            nc.gpsimd.collective_compute(
                "AllToAll", mybir.AluOpType.bypass, replica_groups=rg,
                ins=[a2a_in2.opt()], outs=[a2a_out2.opt()])
            do_oproj(0, oall1, 0, 6)
            oall2 = wpool.tile([128, 8, 256], BF16, tag="oall2")
            nc.sync.dma_start(oall2[:],
                              a2a_out2[:].rearrange("s p t -> p s t"))
            do_oproj(1, oall2, 768, 2)

    nc.compile()
    return nc


def _host_inputs(x, Wq, Wk, Wv, Wo, token_positions):
    """Per-core in_maps with transposed/tiled layouts."""
    x = np.asarray(x, dtype=np.float32)
    xt_bf = np.ascontiguousarray(
        x.reshape(NT, D).T.reshape(8, 128, NT).transpose(1, 0, 2)
    ).astype(ml_dtypes.bfloat16)

    pos = np.asarray(token_positions).astype(np.float64)
    inv_freq = 1.0 / (THETA ** (np.arange(0, DH, 2, dtype=np.float64) / DH))
    ang = pos[None, :] * inv_freq[:, None]          # [32, T]
    cos_p = np.cos(ang)                              # pair i
    sin_p = np.sin(ang)
    # partition p (0..127): within-head dim d = p % 64, pair = d // 2
    d_idx = (np.arange(128) % 64) // 2
    cosb = cos_p[d_idx, :].astype(np.float32)
    sinb = sin_p[d_idx, :].astype(np.float32)

    rotm = np.zeros((128, 128), dtype=np.float32)
    for i in range(64):
        rotm[2 * i + 1, 2 * i] = -1.0   # out[2i] -= in[2i+1]*sin -> rot[2i] = -in[2i+1]
        rotm[2 * i, 2 * i + 1] = 1.0    # rot[2i+1] = in[2i]
    tri = np.tril(np.ones((128, 128), dtype=np.float32)).T  # [tk, tq] tk<=tq
    tri = tri.astype(ml_dtypes.bfloat16)

    def wtiles(W, sl):
        # lhsT tiles: [p, ko, e] with d = ko*128+p contracting
        Wt = np.ascontiguousarray(W[sl, :].T)        # [D, e]
        return np.ascontiguousarray(
            Wt.reshape(8, 128, Wt.shape[1]).transpose(1, 0, 2))

    WoT = np.ascontiguousarray(np.asarray(Wo, dtype=np.float32).T)  # [e_in, e_out]
    wo_t = np.ascontiguousarray(WoT.reshape(8, 128, D).transpose(1, 0, 2))

    in_maps = []
    for c in range(N_CORES):
        sl = slice(EC * c, EC * (c + 1))
        in_maps.append({
            "xt": xt_bf,
            "wq": wtiles(np.asarray(Wq, np.float32), sl).astype(ml_dtypes.bfloat16),
            "wk": wtiles(np.asarray(Wk, np.float32), sl).astype(ml_dtypes.bfloat16),
            "wv": wtiles(np.asarray(Wv, np.float32), sl).astype(ml_dtypes.bfloat16),
            "wo": wo_t.astype(ml_dtypes.bfloat16),
            "cosb": cosb,
            "sinb": sinb,
            "rotm": rotm,
            "trimask": tri,
        })
    return in_maps


def kernel(x, Wq, Wk, Wv, Wo, token_positions):
    global last_results
    if "nc" not in _CACHE:
        _CACHE["nc"] = _build_program()
    nc = _CACHE["nc"]
    in_maps = _host_inputs(x, Wq, Wk, Wv, Wo, token_positions)
    res = bass_utils.run_bass_kernel_spmd(nc, in_maps, list(range(N_CORES)))
    last_results = res
    y = np.empty((NT, D), dtype=np.float32)
    for c in range(N_CORES):
        yc = res.results[c]["y"]
        y[768 * c:768 * c + 768] = yc[0:768]
        y[6144 + 256 * c:6144 + 256 * c + 256] = yc[768:1024]
    return y.reshape(B, T, D)



# revision 31
# speedup vs baseline: 1.5108x; 1.5108x over previous
"""Multi-head self-attention (RoPE, causal) on 8 Trainium2 NeuronCores.

v3: tensor-parallel over heads (2 heads/core). Design notes:

  - The PE array must stream continuously or the HAM activity monitor
    throttles it to 1.2 GHz (v1 spent ~70% of its span cold). Attention
    is processed in 512-wide tq chunks at tk block-PAIR granularity;
    projection / output-projection matmul "filler" quanta are
    interleaved into the attention slots so the PE never waits on ACT,
    with harmless dummy matmuls padding any leftover slots.
  - One exp instruction covers a whole (2 tk-blocks x 2 heads) score
    tile -- ACT pays ~250ns fixed access latency per instruction, so
    fewer/wider exps matter.
  - Causal masking: exp runs unmasked (scores are bounded), then a
    GPSIMD affine_select zeroes the upper triangle of diagonal blocks;
    the stale lane of a diagonal pair is memset to 0.
  - exp -> PV pipelined with a 2-pair lag; PV is bf16 (fp8 DoubleRow
    works but costs ~2e-2 rel err, over the gate).
  - Per-batch AllToAll (head-sharded -> token-sharded) so the output
    projection of batch b fills attention slots of batch b+1; batch 3
    is split c4 0-2 / c4 3 so only a 64-token-per-core A2A + oproj
    remain as the tail.
  - Softmax normalization: fast PSUM drain to SBUF, then
    reciprocal_approx_fast + gpsimd partition_broadcast + one bf16
    multiply.
"""

import numpy as np
import ml_dtypes

import concourse.bacc as bacc
import concourse.mybir as mybir
import concourse.tile as tile
from concourse import bass_utils

F32 = mybir.dt.float32
F32R = mybir.dt.float32r
BF16 = mybir.dt.bfloat16
FP8 = mybir.dt.float8e4

B, T, D = 4, 2048, 1024
H, DH = 16, 64
N_CORES = 8
HPC = H // N_CORES            # heads per core = 2
EC = HPC * DH                 # feature slice per core = 128
NT = B * T                    # 8192 tokens
TPC = NT // N_CORES           # 1024 tokens per core
THETA = 10000.0
TQC = 512                     # tq chunk width
NC4 = T // TQC                # 4 tq chunks per batch
NBP = T // 256                # 8 tk block-pairs per batch
TPB = T // N_CORES            # 256 tokens per core per batch

_CACHE = {}
last_results = None
DEBUG = False

AOP = mybir.AluOpType


def _build_program():
    nc = bacc.Bacc("TRN2", debug=False, target_bir_lowering=False,
                   num_devices=N_CORES)

    xt_d = nc.dram_tensor("xt", [128, 8, NT], BF16, kind="ExternalInput")
    wq_d = nc.dram_tensor("wq", [128, 8, EC], BF16, kind="ExternalInput")
    wk_d = nc.dram_tensor("wk", [128, 8, EC], BF16, kind="ExternalInput")
    wv_d = nc.dram_tensor("wv", [128, 8, EC], BF16, kind="ExternalInput")
    wo_d = nc.dram_tensor("wo", [128, 8, D], BF16, kind="ExternalInput")
    cos_d = nc.dram_tensor("cosb", [128, T], BF16, kind="ExternalInput")
    sin_d = nc.dram_tensor("sinb", [128, T], BF16, kind="ExternalInput")
    rotm_d = nc.dram_tensor("rotm", [128, 128], BF16, kind="ExternalInput")
    idb_d = nc.dram_tensor("identb", [128, 128], BF16, kind="ExternalInput")
    y_d = nc.dram_tensor("y", [TPC, D], F32, kind="ExternalOutput")
    if DEBUG:
        qT_dbg = nc.dram_tensor("qT_dbg", [128, NT], BF16,
                                kind="ExternalOutput")
        kT_dbg = nc.dram_tensor("kT_dbg", [128, NT], BF16,
                                kind="ExternalOutput")
        vx_dbg = nc.dram_tensor("vx_dbg", [128, HPC * B, NBP, 2, 80],
                                BF16, kind="ExternalOutput")
        den_dbg = nc.dram_tensor("den_dbg", [2, NC4, 512], F32,
                                 kind="ExternalOutput")
        ex_dbg = nc.dram_tensor("ex_dbg", [128, 2, 2, 512],
                                BF16, kind="ExternalOutput")

    with tile.TileContext(nc) as tc:
        with (
            tc.tile_pool(name="consts", bufs=1) as consts,
            tc.tile_pool(name="wpool", bufs=1) as wpool,
            tc.tile_pool(name="big", bufs=1) as big,
            tc.tile_pool(name="xp", bufs=3) as xp,
            tc.tile_pool(name="stage", bufs=3) as stage,
            tc.tile_pool(name="expp", bufs=4) as expp,
            tc.tile_pool(name="aop", bufs=4) as aop,
            tc.tile_pool(name="outp", bufs=2) as outp,
            tc.tile_pool(name="sp", bufs=1, space="PSUM") as sp,
            tc.tile_pool(name="pv0", bufs=1, space="PSUM") as pv0,
            tc.tile_pool(name="pv1", bufs=1, space="PSUM") as pv1,
            tc.tile_pool(name="fp", bufs=2, space="PSUM") as fp,
            tc.tile_pool(name="dram", bufs=2, space="DRAM") as dram,
        ):
            pvp = [pv0, pv1]
            # ---- constants ----
            cos_sb = consts.tile([128, T], BF16)
            sin_sb = consts.tile([128, T], BF16)
            rotm_sb = consts.tile([128, 128], BF16)
            identb_sb = consts.tile([128, 128], BF16)
            wq_sb = consts.tile([128, 8, EC], BF16)
            wk_sb = consts.tile([128, 8, EC], BF16)
            wv_sb = consts.tile([128, 8, EC], BF16)
            # weights first: the first projection only needs wq + xt(0)
            nc.sync.dma_start(wq_sb[:], wq_d[:, :, :])
            nc.sync.dma_start(wk_sb[:], wk_d[:, :, :])
            nc.sync.dma_start(wv_sb[:], wv_d[:, :, :])
            nc.sync.dma_start(cos_sb[:], cos_d[:, :])
            nc.sync.dma_start(sin_sb[:], sin_d[:, :])
            nc.sync.dma_start(rotm_sb[:], rotm_d[:, :])
            nc.sync.dma_start(identb_sb[:], idb_d[:, :])

            # ---- persistent tensors ----
            qT = big.tile([128, NT], BF16, tag="qT")
            kT = big.tile([128, NT], BF16, tag="kT")
            # V transposed: [tk-in-block, pair(b*2+h), blockpair, parity, 80]
            vext = big.tile([128, HPC * B, NBP, 2, 80], BF16, tag="vext")
            nc.vector.memset(vext[:, :, :, :, 64], 1.0)

            a2a_in = [dram.tile([N_CORES, 128, TPB], BF16, tag=f"ai{b}",
                                name=f"a2a_in{b}") for b in range(B)]
            a2a_out = [dram.tile([N_CORES, 128, TPB], BF16, tag=f"ao{b}",
                                 name=f"a2a_out{b}") for b in range(B)]
            # batch 3 is all-to-all'd in two halves (tokens 0:1024 /
            # 1024:2048, 128 tokens per core each) so the first half
            # overlaps the tail of attention
            a2a_in3 = [dram.tile([N_CORES, 128, w], BF16, tag=f"ai3{h}",
                                 name=f"a2a_in3{h}")
                       for h, w in ((0, 192), (1, 64))]
            a2a_out3 = [dram.tile([N_CORES, 128, w], BF16, tag=f"ao3{h}",
                                  name=f"a2a_out3{h}")
                        for h, w in ((0, 192), (1, 64))]

            xts = {}

            def dma_chunk(ci):
                if ci < 16 and ci not in xts:
                    xt = xp.tile([128, 8, 512], BF16, tag="x",
                                 name=f"x{ci}")
                    nc.gpsimd.dma_start(xt[:],
                                        xt_d[:, :, 512 * ci:512 * ci + 512])
                    xts[ci] = xt

            # ---------- projection chunk -> list of filler quanta --------
            def chunk_quanta(ci):
                t0 = 512 * ci
                bb = t0 // T
                s0 = t0 % T
                st = {}

                def _proj(w_sb, lohi):
                    def f():
                        if lohi == 0:
                            st["acc"] = fp.tile([128, 512], F32, tag="f",
                                                name=f"p{ci}")
                        pt = st["acc"]
                        for ko in range(4 * lohi, 4 * lohi + 4):
                            nc.tensor.matmul(pt, w_sb[:, ko, :],
                                             xts[ci][:, ko, :],
                                             start=(ko == 0), stop=(ko == 7))
                    return f

                def _copy(nm):
                    def f():
                        raw = stage.tile([128, 512], BF16, tag=nm,
                                         name=f"{nm}{ci}")
                        st[nm] = raw
                        nc.vector.tensor_copy(raw[:], st["acc"][:])
                    return f

                def _rot(nm):
                    def f():
                        rot = fp.tile([128, 512], F32, tag="f",
                                      name=f"r{nm}{ci}")
                        st["rot" + nm] = rot
                        nc.tensor.matmul(rot, rotm_sb[:], st[nm][:],
                                         start=True, stop=True)
                    return f

                def _combine(nm, dest):
                    def f():
                        t1 = stage.tile([128, 512], BF16, tag="t1")
                        nc.vector.tensor_tensor(
                            t1[:], st[nm][:], cos_sb[:, s0:s0 + 512], AOP.mult)
                        t2 = stage.tile([128, 512], BF16, tag="t2")
                        nc.vector.tensor_tensor(
                            t2[:], st["rot" + nm][:], sin_sb[:, s0:s0 + 512],
                            AOP.mult)
                        nc.vector.tensor_tensor(
                            dest[:, t0:t0 + 512], t1[:], t2[:], AOP.add)
                    return f

                def _vtrans(h):
                    def f():
                        pair = bb * HPC + h
                        tp = fp.tile([128, 4, 64], BF16, tag="f",
                                     name=f"vt{ci}{h}")
                        for bi in range(4):
                            nc.tensor.transpose(
                                tp[:, bi, :],
                                st["vraw"][64 * h:64 * h + 64,
                                           128 * bi:128 * bi + 128],
                                identb_sb[64 * h:64 * h + 64,
                                          64 * h:64 * h + 64])
                        jb0 = s0 // 128
                        nc.vector.tensor_copy(
                            vext[:, pair, jb0 // 2:jb0 // 2 + 2, :, 0:64],
                            tp[:].rearrange("p (m c) d -> p m c d", c=2))
                    return f

                def q1():
                    dma_chunk(ci + 1)
                    dma_chunk(ci + 2)
                    _proj(wq_sb, 0)()
                return [q1,
                        lambda: (_proj(wq_sb, 1)(), _copy("rq")()),
                        _proj(wk_sb, 0),
                        lambda: (_proj(wk_sb, 1)(), _copy("rk")()),
                        lambda: (_rot("rq")(), _proj(wv_sb, 0)()),
                        lambda: (_proj(wv_sb, 1)(), _copy("vraw")(),
                                 _combine("rq", qT)()),
                        lambda: (_rot("rk")(), _vtrans(0)()),
                        lambda: (_vtrans(1)(), _combine("rk", kT)())]

            def dummy_quantum():
                ot = fp.tile([128, 512], F32, tag="f", name="dmy")
                nc.tensor.matmul(ot, wq_sb[:, 0, :], qT[:, 0:512],
                                 start=True, stop=True)

            # -------------------- attention ------------------------------
            def run_attn(bb, fillers, c4_hooks=None):
                tb0 = bb * T
                qs = [qT[64 * hh:64 * hh + 64, tb0:tb0 + T] for hh in (0, 1)]
                ks = [kT[64 * hh:64 * hh + 64, tb0:tb0 + T] for hh in (0, 1)]
                fi = [0]

                def fill():
                    if fi[0] < len(fillers):
                        f = fillers[fi[0]]
                        fi[0] += 1
                        if f is not None:
                            f()
                    else:
                        dummy_quantum()

                for c4 in range(NC4):
                    J = 4 * (c4 + 1)
                    M = J // 2
                    pvts = [pvp[hh].tile([65, 512], F32, tag="pv",
                                         name=f"pv{bb}{c4}{hh}")
                            for hh in (0, 1)]
                    exts = {}

                    def do_pv(m, pvts=pvts, M=M):
                        ext, a = exts.pop(m)
                        for hh in (0, 1):
                            for par in (0, 1):
                                nc.tensor.matmul(
                                    pvts[hh][:, a:512],
                                    vext[:, bb * HPC + hh, m, par, 0:65],
                                    ext[:, par, hh, a:512],
                                    start=(m == 0 and par == 0),
                                    stop=(m == M - 1 and par == 1))

                    for m in range(M):
                        if m >= 2:
                            do_pv(m - 2)
                        fill()
                        # scores for the block pair (2 j's x 2 heads) into
                        # one [128, 2, 2, 512] PSUM tile; one exp for all.
                        spt = sp.tile([128, 2, 2, 512], F32, tag="s",
                                      name=f"s{bb}{c4}{m % 2}")
                        lo0 = max(0, 128 * 2 * m - TQC * c4)
                        lo1 = max(0, 128 * (2 * m + 1) - TQC * c4)
                        for par in (0, 1):
                            j = 2 * m + par
                            lo = lo1 if par else lo0
                            for hh in (0, 1):
                                nc.tensor.matmul(
                                    spt[:, par, hh, lo:512],
                                    ks[hh][:, 128 * j:128 * j + 128],
                                    qs[hh][:, TQC * c4 + lo:TQC * c4 + 512],
                                    start=True, stop=True)
                        fill()
                        ext = expp.tile([128, 2, 2, 512], BF16, tag="e",
                                        name=f"e{bb}{c4}{m % 4}")
                        exts[m] = (ext, lo0)
                        if lo1 > lo0:  # diagonal pair: per-plane exp, the
                            # stale lane of plane 1 is memset (disjoint)
                            nc.vector.memset(ext[:, 1, :, lo0:lo1], 0.0)
                            nc.scalar.activation(
                                ext[:, 0, :, lo0:512], spt[:, 0, :, lo0:512],
                                mybir.ActivationFunctionType.Exp, scale=0.125)
                            nc.scalar.activation(
                                ext[:, 1, :, lo1:512], spt[:, 1, :, lo1:512],
                                mybir.ActivationFunctionType.Exp, scale=0.125)
                            # zero exp() above the causal diagonal (keep
                            # where col - partition >= 0)
                            for par in (0, 1):
                                lo = lo1 if par else lo0
                                nc.gpsimd.affine_select(
                                    ext[:, par, :, lo:lo + 128],
                                    ext[:, par, :, lo:lo + 128],
                                    pattern=[[0, 2], [1, 128]],
                                    compare_op=AOP.is_ge, fill=0.0,
                                    base=0, channel_multiplier=-1)
                        else:
                            nc.scalar.activation(
                                ext[:, :, :, lo0:512], spt[:, :, :, lo0:512],
                                mybir.ActivationFunctionType.Exp, scale=0.125)
                        if DEBUG and bb == 0 and c4 == 0 and m == 0:
                            nc.sync.dma_start(ex_dbg[:, :, :, :], ext[:])
                    do_pv(M - 2)
                    do_pv(M - 1)
                    fill()
                    # ---- epilogue: drain PSUM fast, normalize, ship ----
                    for hh in (0, 1):
                        unn = aop.tile([64, 512], BF16, tag="unn",
                                       name=f"unn{hh}")
                        nc.vector.tensor_copy(unn[:], pvts[hh][0:64, :])
                        dsb = aop.tile([1, 512], F32, tag="dsb",
                                       name=f"dsb{hh}")
                        nc.vector.tensor_copy(dsb[:], pvts[hh][64:65, :])
                        rec = aop.tile([1, 512], F32, tag="rec",
                                       name=f"rec{hh}")
                        nc.vector.reciprocal_approx_fast(rec[:], dsb[:])
                        if DEBUG and bb == 0:
                            nc.sync.dma_start(den_dbg[hh, c4, :], rec[:])
                        recb = aop.tile([64, 512], F32, tag="recb",
                                        name=f"recb{hh}")
                        nc.gpsimd.partition_broadcast(recb[:], rec[:])
                        ao = aop.tile([64, 512], BF16, tag="ao",
                                      name=f"ao{hh}")
                        nc.vector.tensor_tensor(
                            ao[:], unn[:, :], recb[:], AOP.mult)
                        if bb < 3:
                            grp, tt, W = a2a_in[bb], TQC * c4, TPB
                        elif c4 < 3:
                            grp, tt, W = a2a_in3[0], TQC * c4, 192
                        else:
                            grp, tt, W = a2a_in3[1], 0, 64
                        off = 0
                        while off < 512:
                            dd = (tt + off) // W
                            col = (tt + off) % W
                            w = min(512 - off, W - col)
                            nc.sync.dma_start(
                                grp[dd, 64 * hh:64 * hh + 64,
                                    col:col + w],
                                ao[:, off:off + w])
                            off += w
                        if hh == 0:
                            fill()
                    fill()
                    if c4_hooks and c4 in c4_hooks:
                        c4_hooks[c4]()
                while fi[0] < len(fillers):
                    fill()

            # -------------------- output projection ----------------------
            wos = {}

            def load_wo(eo):
                wo_sb = wpool.tile([128, 8, 512], BF16, tag=f"wo{eo}",
                                   name=f"wo_{eo}")
                nc.gpsimd.dma_start(wo_sb[:],
                                    wo_d[:, :, 512 * eo:512 * eo + 512])
                wos[eo] = wo_sb

            def oproj_quantum(oall_g, row0, eo, tb, nrows=128):
                def f():
                    ot = fp.tile([128, 512], F32, tag="f",
                                 name=f"ot{row0}{eo}{tb}")[0:nrows, :]
                    for ec in range(8):
                        nc.tensor.matmul(
                            ot, oall_g[:, ec, 128 * tb:128 * tb + nrows],
                            wos[eo][:, ec, :],
                            start=(ec == 0), stop=(ec == 7))
                    ys = outp.tile([128, 512], F32, tag="y",
                                   name="ys")[0:nrows, :]
                    nc.scalar.copy(ys[:], ot)
                    nc.sync.dma_start(
                        y_d[row0 + 128 * tb:row0 + 128 * tb + nrows,
                            512 * eo:512 * eo + 512], ys[:])
                return f

            def post_attn(bb):
                """A2A + oall load for batch bb; returns oproj quanta."""
                rg = [list(range(N_CORES))]
                nc.gpsimd.collective_compute(
                    "AllToAll", AOP.bypass, replica_groups=rg,
                    ins=[a2a_in[bb].opt()], outs=[a2a_out[bb].opt()])
                oall = wpool.tile([128, 8, TPB], BF16, tag=f"oall{bb}",
                                  name=f"oall{bb}")
                nc.gpsimd.dma_start(oall[:],
                                    a2a_out[bb][:].rearrange("s p t -> p s t"))
                return [oproj_quantum(oall, TPB * bb, eo, tb)
                        for eo in range(2) for tb in range(2)]

            # ----------------------- main flow ---------------------------
            dma_chunk(0)
            for ci in range(2):
                for q in chunk_quanta(ci):
                    q()
            run_attn(0, [q for ci in range(2, 8)
                         for q in chunk_quanta(ci)])
            op0 = post_attn(0)
            load_wo(0)
            load_wo(1)
            run_attn(1, [q for ci in range(8, 12)
                         for q in chunk_quanta(ci)] + op0)
            op1 = post_attn(1)
            run_attn(2, [q for ci in range(12, 16)
                         for q in chunk_quanta(ci)] + op1)
            op2 = post_attn(2)
            rg = [list(range(N_CORES))]
            fillers3 = [dummy_quantum] * 34 + op2

            def half_a():
                nc.gpsimd.collective_compute(
                    "AllToAll", AOP.bypass, replica_groups=rg,
                    ins=[a2a_in3[0].opt()], outs=[a2a_out3[0].opt()])
                oall3a = wpool.tile([128, 8, 192], BF16, tag="oall3a",
                                    name="oall3a")
                nc.gpsimd.dma_start(
                    oall3a[:], a2a_out3[0][:].rearrange("s p t -> p s t"))
                fillers3.extend([dummy_quantum] * 4)
                fillers3.extend(oproj_quantum(oall3a, TPB * 3, eo, tb,
                                              128 if tb == 0 else 64)
                                for eo in range(2) for tb in range(2))

            # guard: give A2A(2) dummy PE work before the first oproj
            # quantum can reach the head of the PE queue
            run_attn(3, fillers3, c4_hooks={2: half_a})
            nc.gpsimd.collective_compute(
                "AllToAll", AOP.bypass, replica_groups=rg,
                ins=[a2a_in3[1].opt()], outs=[a2a_out3[1].opt()])
            oall3b = wpool.tile([128, 8, 64], BF16, tag="oall3b",
                                name="oall3b")
            nc.gpsimd.dma_start(
                oall3b[:], a2a_out3[1][:].rearrange("s p t -> p s t"))
            if DEBUG:
                nc.sync.dma_start(qT_dbg[:, :], qT[:])
                nc.sync.dma_start(kT_dbg[:, :], kT[:])
                nc.sync.dma_start(vx_dbg[:, :, :, :, :], vext[:])
            for eo in range(2):
                oproj_quantum(oall3b, TPB * 3 + 192, eo, 0, 64)()

    nc.compile()
    return nc


def _host_inputs(x, Wq, Wk, Wv, Wo, token_positions):
    """Per-core in_maps with transposed/tiled layouts."""
    x = np.asarray(x, dtype=np.float32)
    xt_bf = np.ascontiguousarray(
        x.reshape(NT, D).T.reshape(8, 128, NT).transpose(1, 0, 2)
    ).astype(ml_dtypes.bfloat16)

    pos = np.asarray(token_positions).astype(np.float64)
    inv_freq = 1.0 / (THETA ** (np.arange(0, DH, 2, dtype=np.float64) / DH))
    ang = pos[None, :] * inv_freq[:, None]          # [32, T]
    cos_p = np.cos(ang)
    sin_p = np.sin(ang)
    d_idx = (np.arange(128) % 64) // 2
    cosb = cos_p[d_idx, :].astype(ml_dtypes.bfloat16)
    sinb = sin_p[d_idx, :].astype(ml_dtypes.bfloat16)

    rotm = np.zeros((128, 128), dtype=np.float32)
    for i in range(64):
        rotm[2 * i + 1, 2 * i] = -1.0
        rotm[2 * i, 2 * i + 1] = 1.0
    rotm = rotm.astype(ml_dtypes.bfloat16)
    identb = np.eye(128, dtype=np.float32).astype(ml_dtypes.bfloat16)

    def wtiles(W, sl):
        Wt = np.ascontiguousarray(W[sl, :].T)        # [D, e]
        return np.ascontiguousarray(
            Wt.reshape(8, 128, Wt.shape[1]).transpose(1, 0, 2))

    WoT = np.ascontiguousarray(np.asarray(Wo, dtype=np.float32).T)
    wo_t = np.ascontiguousarray(WoT.reshape(8, 128, D).transpose(1, 0, 2))

    in_maps = []
    for c in range(N_CORES):
        sl = slice(EC * c, EC * (c + 1))
        in_maps.append({
            "xt": xt_bf,
            "wq": wtiles(np.asarray(Wq, np.float32), sl).astype(ml_dtypes.bfloat16),
            "wk": wtiles(np.asarray(Wk, np.float32), sl).astype(ml_dtypes.bfloat16),
            "wv": wtiles(np.asarray(Wv, np.float32), sl).astype(ml_dtypes.bfloat16),
            "wo": wo_t.astype(ml_dtypes.bfloat16),
            "cosb": cosb,
            "sinb": sinb,
            "rotm": rotm,
            "identb": identb,
        })
    return in_maps


def _assemble_output(res):
    y = np.empty((B, T, D), dtype=np.float32)
    for c in range(N_CORES):
        yc = res.results[c]["y"]          # [4*256, D]
        for b in range(3):
            y[b, TPB * c:TPB * c + TPB] = yc[TPB * b:TPB * b + TPB]
        # batch 3 arrives in two parts: 192 tokens (c4 0-2) + 64 (c4 3)
        y[3, 192 * c:192 * c + 192] = yc[768:960]
        y[3, 1536 + 64 * c:1536 + 64 * c + 64] = yc[960:1024]
    return y


def kernel(x, Wq, Wk, Wv, Wo, token_positions):
    global last_results
    if "nc" not in _CACHE:
        _CACHE["nc"] = _build_program()
    nc = _CACHE["nc"]
    in_maps = _host_inputs(x, Wq, Wk, Wv, Wo, token_positions)
    res = bass_utils.run_bass_kernel_spmd(nc, in_maps, list(range(N_CORES)))
    last_results = res
    return _assemble_output(res)
